# revision 4
# baseline (speedup 1.0000x reference)
"""LIF neuron step on 8 Trainium2 NeuronCores.

Math (reference):
    I_raw   = g @ w                       # [N] vec-mat product, w is [N, N]
    I       = sigmoid(12/N * I_raw) + 0.9 * x_in
    v_next  = v + (E_L - v + I * (30 - E_L)) / tau_m
    out     = sigmoid(v_next - 30)

Everything after the matvec is affine in I_sig = sigmoid(12/N * I_raw):
    out = sigmoid(B * (I_sig + D/B)),  B = (30 - E_L)/tau_m (uniform scalar),
    D   = v + (E_L - v)/tau_m - 30 + 0.9 * x_in * B  (per-neuron, host-computed)

Sharding: w is split column-wise (output-neuron dim) into 8 shards of
[8192, 1024]; g is replicated. Each core computes its 1024 outputs fully
locally; host concatenates.

The kernel is memory-bound on streaming the w shard, so w/g are cast to
fp8 e4m3 on the host (the matvec averages 8192 near-independent products,
so quantization noise mostly cancels; measured output rel-err ~9e-3 vs
the 2e-2 gate).  The host pre-arranges the shard in the exact SBUF layout
([p, t, c], k = t*128 + p) so every chunk DMA is one contiguous run per
partition.

PE structure: g is the STATIONARY operand ([128, 2, 1] fp8 pairs) and w
the MOVING operand ([128, 2, 512] slabs) with perf_mode=DoubleRow, which
streams 256 w elements/cycle.  This needs only ~100 PE instructions
(vs ~1000 for per-tile LDWEIGHTS), which keeps the NEFF instruction
stream to a single iram page — instruction paging otherwise rides DMA
engine 0 mid-stream and delays the final chunks by several us.  The
per-core result accumulates as two [1, 512] fp32 PSUM tiles; the tail is
2 ACT sigmoids + 1 DVE add on partition 0, then a single-descriptor
4 KB output DMA.
"""

from contextlib import ExitStack

import numpy as np

import concourse.bass as bass
import concourse.bacc as bacc
import concourse.mybir as mybir
import concourse.tile as tile
from concourse.bass_utils import run_bass_kernel_spmd

N = 8192          # neurons
NCORES = 8
COLS = N // NCORES  # 1024 output neurons per core
P = 128           # partitions
KT = N // P       # 64 contraction tiles of 128
SKT = KT // 2     # 32 super k-tiles of 256 (DoubleRow)
# super-ktiles per DMA chunk: small first chunk so PE starts early, small
# last chunk so PE finishes right behind the final DMA packet.
CHUNK_SIZES = [2, 6, 8, 8, 6, 2]
GPAD = 16         # stationary dim-1 step must be 16B-aligned
SPIKE = 30.0
F8 = mybir.dt.float8e4

TRACE = False          # set True to capture NTFF profile
LAST_RESULT = None     # BassKernelResults of the most recent run

_NC = None
_B_CONST = None


def _build(b_const):
    nc = bacc.Bacc("TRN2", target_bir_lowering=False, debug=False,
                   num_devices=NCORES)
    # host pre-layout: wt[p, t*COLS + c] = w[t*128 + p, c]  (fp8)
    wt = nc.dram_tensor("wt", [P, KT * COLS], F8, kind="ExternalInput").ap()
    # gt[p, t*GPAD] = g[t*128 + p], zero-padded so the DoubleRow stationary
    # AP's middle-dim step is 16 bytes.
    gt = nc.dram_tensor("gt", [P, KT * GPAD], F8, kind="ExternalInput").ap()
    db = nc.dram_tensor("db", [1, COLS], mybir.dt.float32,
                        kind="ExternalInput").ap()
    out = nc.dram_tensor("out", [1, COLS], mybir.dt.float32,
                         kind="ExternalOutput").ap()

    with tile.TileContext(nc) as tc, ExitStack() as ctx:
        wpool = ctx.enter_context(tc.tile_pool(name="w", bufs=1))
        spool = ctx.enter_context(tc.tile_pool(name="s", bufs=1))
        ppool = ctx.enter_context(tc.tile_pool(name="p", bufs=1, space="PSUM"))

        # w chunks stream on the SP HWDGE ring; the small g/db loads go via
        # the ACT HWDGE ring so their descriptor generation doesn't delay
        # the first w packets.
        wsbs = []
        s0 = 0
        for ci, sct in enumerate(CHUNK_SIZES):
            ct = 2 * sct
            wsb = wpool.tile([P, ct * COLS], F8, tag=f"w{ci}")
            nc.sync.dma_start(wsb[:], wt[:, 2 * s0 * COLS:
                                         2 * (s0 + sct) * COLS])
            wsbs.append((s0, sct, wsb))
            s0 += sct

        gsb = spool.tile([P, KT * GPAD], F8)
        nc.scalar.dma_start(gsb[:], gt[:])
        dbsb = spool.tile([1, COLS], mybir.dt.float32)
        nc.scalar.dma_start(dbsb[:], db[:])

        gs3 = gsb[:].rearrange("p (t q) -> p t q", q=GPAD)
        acc = [ppool.tile([1, 512], mybir.dt.float32, tag=f"acc{h}",
                          name=f"acc{h}")
               for h in range(2)]
        for s0, sct, wsb in wsbs:
            ws3 = wsb[:].rearrange("p (t c) -> p t c", c=COLS)
            for sl in range(sct):
                s = s0 + sl
                for h in range(2):
                    nc.tensor.matmul(
                        acc[h][:, :],
                        gs3[:, 2 * s:2 * s + 2, 0:1],
                        ws3[:, 2 * sl:2 * sl + 2, 512 * h:512 * (h + 1)],
                        start=(s == 0),
                        stop=(s == SKT - 1),
                        perf_mode=mybir.MatmulPerfMode.DoubleRow,
                    )

        # Tail: out = sigmoid(B * (sigmoid(acc*12/N) + D/B)); B is uniform
        # so it rides the second ACT's scalar scale; D/B is per-neuron and
        # is added with one DVE tensor-tensor op.
        isig = spool.tile([1, COLS], mybir.dt.float32)
        for h in range(2):
            nc.scalar.activation(isig[:, 512 * h:512 * (h + 1)], acc[h][:, :],
                                 mybir.ActivationFunctionType.Sigmoid,
                                 scale=12.0 / N)
        aff = spool.tile([1, COLS], mybir.dt.float32)
        nc.vector.tensor_add(aff[:], isig[:], dbsb[:])
        res = spool.tile([1, COLS], mybir.dt.float32)
        nc.scalar.activation(res[:], aff[:],
                             mybir.ActivationFunctionType.Sigmoid,
                             scale=float(b_const))
        nc.sync.dma_start(out[:], res[:])
    nc.compile()
    return nc


def make_in_maps(x_in, v, g, w, E_L, tau_m, b_const):
    np8 = mybir.dt.np(F8)
    w8 = np.asarray(w, dtype=np.float32).astype(np8)
    g8 = np.asarray(g, dtype=np.float32).astype(np8)
    gt = np.zeros((P, KT * GPAD), dtype=np8)
    gt[:, ::GPAD] = g8.reshape(KT, P).T

    E = np.asarray(E_L, dtype=np.float64)
    TM = np.asarray(tau_m, dtype=np.float64)
    V = np.asarray(v, dtype=np.float64)
    X = np.asarray(x_in, dtype=np.float64)
    D = V + (E - V) / TM - SPIKE + 0.9 * X * b_const
    DB = (D / b_const).astype(np.float32)

    in_maps = []
    for c in range(NCORES):
        sl = slice(c * COLS, (c + 1) * COLS)
        # [p, t, c] layout: partition p holds k-tiles t=0..KT-1 contiguously
        wtc = np.ascontiguousarray(
            w8[:, sl].reshape(KT, P, COLS).transpose(1, 0, 2)
        ).reshape(P, KT * COLS)
        in_maps.append({
            "wt": wtc,
            "gt": gt,
            "db": DB[sl].reshape(1, COLS),
        })
    return in_maps


def kernel(x_in, v, g, w, E_L, tau_m, tau_g=None, **_unused):
    global _NC, _B_CONST, LAST_RESULT
    B = (SPIKE - np.asarray(E_L, dtype=np.float64)) \
        / np.asarray(tau_m, dtype=np.float64)
    b_const = float(B[0])
    assert np.allclose(B, b_const, rtol=1e-6), \
        "kernel assumes uniform E_L/tau_m"
    if _NC is None or _B_CONST != b_const:
        _NC = _build(b_const)
        _B_CONST = b_const
    in_maps = make_in_maps(x_in, v, g, w, E_L, tau_m, b_const)
    LAST_RESULT = run_bass_kernel_spmd(_NC, in_maps, list(range(NCORES)),
                                       trace=TRACE)
    out = np.empty(N, dtype=np.float32)
    for c in range(NCORES):
        out[c * COLS:(c + 1) * COLS] = LAST_RESULT.results[c]["out"][0]
    return out
